# revision 11
# baseline (speedup 1.0000x reference)
"""Causal MHA (B=2, L=2048, D=1024, 16 heads, RoPE) on 8 Trainium2 NeuronCores.

Strategy: tensor-parallel over heads (2 heads/core).
 - Host: transpose x -> x^T, build per-core W_qkv^T slices (q cols pre-scaled by
   1/sqrt(hd)), W_out^T, and RoPE cos/sin tables; everything fp16 for matmul
   operands (fp32 PSUM accumulation on device).
 - Device per core: QK^T projection -> RoPE -> S^T = K^T.T @ Q^T per (batch,
   head) with causal block skipping -> exp (no max subtraction; scores are
   ~N(0,1)) -> P^T fp16 -> h'^T = V'.T @ P^T with a ones-column in V' giving
   the softmax denominator -> normalize -> AllToAll so each core holds full
   h^T for a 512-token chunk -> output projection for that chunk.
 - Host: concatenate the 8 [1024, 512] chunks of out^T, transpose, reshape.
"""

import numpy as np

import concourse.bass as bass
import concourse.mybir as mybir
import concourse.tile as tile
from concourse import bacc
from concourse.bass_utils import run_bass_kernel_spmd

B, L, D, NH, HD = 2, 2048, 1024, 16, 64
ROPE_BASE = 10000.0
N_CORES = 8
HPC = NH // N_CORES          # heads per core = 2
M = B * L                    # 4096 tokens
MCH = 512                    # m-chunk (proj free dim)
NMC = M // MCH               # 8
KT = D // 128                # 8 contraction tiles
QB = 512                     # q chunk in attention
KB = 128                     # k block in attention
NKB = L // KB                # 16
NQC = L // QB                # 4

fp16 = mybir.dt.float16
fp32 = mybir.dt.float32

_NC = None


def _build_nc():
    nc = bacc.Bacc("TRN2", target_bir_lowering=False, debug=False,
                   num_devices=N_CORES)

    xT = nc.dram_tensor("xT", [D, M], fp16, kind="ExternalInput").ap()
    wqkvT = nc.dram_tensor("wqkvT", [D, 384], fp16, kind="ExternalInput").ap()
    woutT = nc.dram_tensor("woutT", [D, D], fp16, kind="ExternalInput").ap()
    cosT = nc.dram_tensor("cosT", [128, M], fp16, kind="ExternalInput").ap()
    sinT = nc.dram_tensor("sinT", [128, M], fp16, kind="ExternalInput").ap()
    out = nc.dram_tensor("out", [D, MCH], fp32, kind="ExternalOutput").ap()

    cc_in = nc.dram_tensor("cc_in", [D, MCH], fp16)
    cc_out = nc.dram_tensor("cc_out", [D, MCH], fp16)

    with tile.TileContext(nc) as tc:
        with tc.tile_pool(name="persist", bufs=1) as per, \
             tc.tile_pool(name="weights", bufs=1) as wp:
            # persistent SBUF: inputs, tables, Q/K units, V', masks
            xt = [per.tile([128, M], fp16, tag=f"xt{k}", name=f"xt{k}") for k in range(KT)]
            for k in range(KT):
                nc.sync.dma_start(xt[k][:], xT[k * 128:(k + 1) * 128, :])
            wq = [wp.tile([128, 384], fp16, tag=f"wq{k}", name=f"wq{k}") for k in range(KT)]
            for k in range(KT):
                nc.sync.dma_start(wq[k][:], wqkvT[k * 128:(k + 1) * 128, :])
            wo = [wp.tile([128, D], fp16, tag=f"wo{k}", name=f"wo{k}") for k in range(KT)]
            for k in range(KT):
                nc.sync.dma_start(wo[k][:], woutT[k * 128:(k + 1) * 128, :])
            cos_t = per.tile([128, M], fp16, tag="cos")
            sin_t = per.tile([128, M], fp16, tag="sin")
            nc.sync.dma_start(cos_t[:], cosT[:])
            nc.sync.dma_start(sin_t[:], sinT[:])

            # triangular 0/1 mask: keep where q-col >= k-row
            mask0 = per.tile([128, QB], fp16, tag="mask0")
            nc.gpsimd.memset(mask0[:], 1.0)
            nc.gpsimd.affine_select(
                out=mask0[:], in_=mask0[:], compare_op=mybir.AluOpType.is_ge,
                fill=0.0, base=0, channel_multiplier=-1, pattern=[[1, QB]],
            )

            # Q^T/K^T per batch: rows 0-63 head0, 64-127 head1;
            # cols 0:L = Q, L:2L = K
            qku = [per.tile([128, 2 * L], fp16, tag=f"qku{b}", name=f"qku{b}") for b in range(B)]
            # V' per batch: per k-tile 130 cols = [v_h0(64) | 1 | v_h1(64) | 1]
            vt = [per.tile([128, (L // 128) * 130], fp16, tag=f"vt{b}", name=f"vt{b}")
                  for b in range(B)]
            for b in range(B):
                nc.gpsimd.memset(vt[b][:], 1.0)

            # ---- Phase 1: QK projection + RoPE ----
            with tc.tile_pool(name="qkp_ps", bufs=2, space="PSUM") as qkps, \
                 tc.tile_pool(name="rope_sb", bufs=3) as rsb:
                for b_p in range(B):
                    for lh in range(HPC):
                        for mcb in range(NQC):
                            mc = b_p * NQC + mcb
                            qkp = qkps.tile([128, MCH], fp32, tag="qkp")
                            for k in range(KT):
                                nc.tensor.matmul(
                                    qkp[:], wq[k][:, lh * 128:(lh + 1) * 128],
                                    xt[k][:, mc * MCH:(mc + 1) * MCH],
                                    start=(k == 0), stop=(k == KT - 1))
                            qk16 = rsb.tile([128, MCH], fp16, tag="qk16")
                            nc.scalar.copy(qk16[:], qkp[:])
                            a_t = rsb.tile([128, MCH], fp16, tag="a")
                            c_t = rsb.tile([128, MCH], fp16, tag="c")
                            cs = slice(mc * MCH, (mc + 1) * MCH)
                            nc.vector.tensor_mul(a_t[:], qk16[:], cos_t[:, cs])
                            nc.vector.tensor_mul(c_t[:], qk16[:], sin_t[:, cs])
                            tmp = rsb.tile([128, MCH], fp16, tag="tmp")
                            for g in range(4):  # swap 32-row halves
                                src = (g ^ 1) * 32
                                nc.sync.dma_start(tmp[g * 32:(g + 1) * 32, :],
                                                  c_t[src:src + 32, :])
                            bcol = mcb * MCH
                            # head0: q rows 0-63, k rows 64-127
                            # head1: k rows 0-63, q rows 64-127
                            qrows = slice(0, 64) if lh == 0 else slice(64, 128)
                            krows = slice(64, 128) if lh == 0 else slice(0, 64)
                            drows = slice(lh * 64, (lh + 1) * 64)
                            nc.vector.tensor_add(
                                qku[b_p][drows, bcol:bcol + MCH],
                                a_t[qrows, :], tmp[qrows, :])
                            nc.vector.tensor_add(tmp[krows, :], a_t[krows, :],
                                                 tmp[krows, :])
                            nc.sync.dma_start(
                                qku[b_p][drows, L + bcol:L + bcol + MCH],
                                tmp[krows, :])

            # ---- Phase 2: V projection (natural layout) ----
            with tc.tile_pool(name="v_ps", bufs=3, space="PSUM") as vps:
                for mt in range(M // 128):
                    vp = vps.tile([128, 128], fp32, tag="vp")
                    for k in range(KT):
                        nc.tensor.matmul(
                            vp[:], xt[k][:, mt * 128:(mt + 1) * 128],
                            wq[k][:, 256:384],
                            start=(k == 0), stop=(k == KT - 1))
                    b_, kt_ = mt // (L // 128), mt % (L // 128)
                    # strided dst: two 64-col blocks at +0 and +65
                    dst = vt[b_][:, kt_ * 130:kt_ * 130 + 130]
                    dst = dst.rearrange("p (g c) -> p g c", g=2)[:, :, 0:64]
                    nc.scalar.copy(dst, vp[:].rearrange("p (g c) -> p g c", g=2))

            # ---- Phase 3: attention per (batch, head) ----
            # Two heads of a batch interleaved; q-chunks in pairs so hacc
            # needs 2 banks/head and ST pairs share one [128,1024] tile
            # with a single exp per (head, ki).
            with tc.tile_pool(name="st_ps", bufs=2, space="PSUM") as stps, \
                 tc.tile_pool(name="h_ps", bufs=4, space="PSUM") as hps, \
                 tc.tile_pool(name="att_sb", bufs=4) as asb, \
                 tc.tile_pool(name="norm_sb", bufs=4) as nsb:
                for b_ in range(B):
                    for pas in range(2):
                        qcs = (2 * pas, 2 * pas + 1)
                        kmax = (qcs[1] + 1) * (QB // KB)
                        hacc = {}
                        for lh in range(HPC):
                            for qc in qcs:
                                hacc[(lh, qc)] = hps.tile(
                                    [65, QB], fp32, tag="hacc", name="hacc")
                        for ki in range(kmax):
                            qlo = max(qcs[0], ki // (QB // KB))
                            pofs = []
                            for qc in range(qlo, qcs[1] + 1):
                                diag = (qc == ki // (QB // KB))
                                off = (ki % (QB // KB)) * KB if diag else 0
                                w = QB - off
                                # fixed 512-stride slots so each matmul stays
                                # inside one PSUM bank; diag slack unread
                                pofs.append((qc, (qc - qlo) * QB, w,
                                             qc * QB + off, off))
                            p0 = (qcs[1] + 1 - qlo) * QB
                            pts = {}
                            for lh in range(HPC):
                                rows = slice(lh * 64, (lh + 1) * 64)
                                st = stps.tile([128, p0], fp32, tag="st")
                                pt = asb.tile([128, p0], fp16, tag="pt")
                                pts[lh] = pt
                                for qc, ps, w, qs, off in pofs:
                                    nc.tensor.matmul(
                                        st[:, ps:ps + w],
                                        qku[b_][rows,
                                                L + ki * KB:L + (ki + 1) * KB],
                                        qku[b_][rows, qs:qs + w],
                                        start=True, stop=True)
                                nc.scalar.activation(
                                    pt[:], st[:],
                                    mybir.ActivationFunctionType.Exp)
                                if pofs[0][4] or ki // (QB // KB) == qlo:
                                    w0 = pofs[0][2]
                                    nc.vector.tensor_mul(
                                        pt[:, 0:w0], pt[:, 0:w0],
                                        mask0[:, 0:w0])
                            for lh in range(HPC):
                                vsl = vt[b_][:, ki * 130 + lh * 65:
                                             ki * 130 + lh * 65 + 65]
                                for qc, ps, w, qs, off in pofs:
                                    nc.tensor.matmul(
                                        hacc[(lh, qc)][:, off:off + w], vsl,
                                        pts[lh][:, ps:ps + w],
                                        start=(ki == 0),
                                        stop=(ki == (qc + 1) * (QB // KB) - 1))
                        # normalize + ship to cc_in
                        for lh in range(HPC):
                            for qc in qcs:
                                ha = hacc[(lh, qc)]
                                dsb = nsb.tile([1, QB], fp32, tag="dsb")
                                nc.scalar.copy(dsb[:], ha[64:65, :])
                                recip = nsb.tile([1, QB], fp32, tag="recip")
                                nc.vector.reciprocal_approx_fast(
                                    recip[:], dsb[:])
                                rb = nsb.tile([64, QB], fp32, tag="rb")
                                nc.gpsimd.partition_broadcast(rb[:], recip[:])
                                ht = nsb.tile([64, QB], fp16, tag="ht")
                                nc.vector.tensor_mul(
                                    ht[:], ha[0:64, :], rb[:])
                                j = b_ * NQC + qc
                                nc.sync.dma_start(
                                    cc_in.ap()[j * 128 + lh * 64:
                                               j * 128 + lh * 64 + 64, :],
                                    ht[:])

            # ---- Phase 4: AllToAll + output projection ----
            nc.gpsimd.collective_compute(
                "AllToAll", mybir.AluOpType.bypass,
                replica_groups=[list(range(N_CORES))],
                ins=[cc_in.ap().opt()], outs=[cc_out.ap().opt()],
            )
            with tc.tile_pool(name="op_ps", bufs=2, space="PSUM") as ops, \
                 tc.tile_pool(name="op_sb", bufs=3) as osb:
                htf = [osb.tile([128, MCH], fp16, tag=f"htf{k}", name=f"htf{k}")
                       for k in range(KT)]
                for k in range(KT):
                    nc.sync.dma_start(htf[k][:],
                                      cc_out.ap()[k * 128:(k + 1) * 128, :])
                for eb in range(KT):
                    op = ops.tile([128, MCH], fp32, tag="op")
                    for k in range(KT):
                        nc.tensor.matmul(
                            op[:], wo[k][:, eb * 128:(eb + 1) * 128], htf[k][:],
                            start=(k == 0), stop=(k == KT - 1))
                    ot = osb.tile([128, MCH], fp32, tag="ot")
                    nc.scalar.copy(ot[:], op[:])
                    nc.sync.dma_start(out[eb * 128:(eb + 1) * 128, :], ot[:])

    nc.compile()
    return nc


def _host_inputs(x, Wqkv, Wout):
    """Build the 8 per-core input maps (all fp16)."""
    x = np.asarray(x, dtype=np.float32)
    Wqkv = np.asarray(Wqkv, dtype=np.float32)
    Wout = np.asarray(Wout, dtype=np.float32)

    xT = np.ascontiguousarray(x.reshape(M, D).T).astype(np.float16)
    woutT = np.ascontiguousarray(Wout.T).astype(np.float16)

    scale = HD ** -0.5
    inv = ROPE_BASE ** (-np.arange(32, dtype=np.float64) / 32.0)
    l = np.arange(L, dtype=np.float64)
    ang = l[None, :] * inv[:, None]                      # [32, L]
    cos32 = np.cos(ang)
    sin32 = np.sin(ang)
    cosT = np.tile(cos32, (4, B)).astype(np.float16)     # [128, M]
    sgn = np.repeat([1.0, -1.0, 1.0, -1.0], 32)[:, None]
    sinT = (np.tile(sin32, (4, B)) * sgn).astype(np.float16)

    in_maps = []
    for c in range(N_CORES):
        a = HPC * c
        cols = []
        cols.append(Wqkv[HD * a:HD * (a + 1), :] * scale)          # q_a
        cols.append(Wqkv[D + HD * a:D + HD * (a + 1), :])          # k_a
        cols.append(Wqkv[D + HD * (a + 1):D + HD * (a + 2), :])    # k_{a+1}
        cols.append(Wqkv[HD * (a + 1):HD * (a + 2), :] * scale)    # q_{a+1}
        cols.append(Wqkv[2 * D + HD * a:2 * D + HD * (a + 1), :])  # v_a
        cols.append(Wqkv[2 * D + HD * (a + 1):2 * D + HD * (a + 2), :])
        wqkvT = np.ascontiguousarray(np.concatenate(cols, 0).T).astype(np.float16)
        in_maps.append({"xT": xT, "wqkvT": wqkvT, "woutT": woutT,
                        "cosT": cosT, "sinT": sinT})
    return in_maps


def kernel(x, Wqkv, Wout, _trace=False):
    global _NC
    if _NC is None:
        _NC = _build_nc()
    in_maps = _host_inputs(x, Wqkv, Wout)
    res = run_bass_kernel_spmd(_NC, in_maps, core_ids=list(range(N_CORES)),
                               trace=_trace)
    outT = np.concatenate([res.results[c]["out"] for c in range(N_CORES)],
                          axis=1)                        # [D, M]
    full = outT.T.reshape(B, L, D).astype(np.float32)
    if _trace:
        kernel.last_results = res
    return full


# revision 14
# speedup vs baseline: 1.0353x; 1.0353x over previous
"""Causal MHA (B=2, L=2048, D=1024, 16 heads, RoPE) on 8 Trainium2 NeuronCores.

Strategy: tensor-parallel over heads (2 heads/core).
 - Host: transpose x -> x^T, build per-core W_qkv^T slices (q cols pre-scaled by
   1/sqrt(hd)), W_out^T, and RoPE cos/sin tables; everything fp16 for matmul
   operands (fp32 PSUM accumulation on device).
 - Device per core: QK^T projection -> RoPE -> S^T = K^T.T @ Q^T per (batch,
   head) with causal block skipping -> exp (no max subtraction; scores are
   ~N(0,1)) -> P^T fp16 -> h'^T = V'.T @ P^T with a ones-column in V' giving
   the softmax denominator -> normalize -> AllToAll so each core holds full
   h^T for a 512-token chunk -> output projection for that chunk.
 - Host: concatenate the 8 [1024, 512] chunks of out^T, transpose, reshape.
"""

import numpy as np

import concourse.bass as bass
import concourse.mybir as mybir
import concourse.tile as tile
from concourse import bacc
from concourse.bass_utils import run_bass_kernel_spmd

B, L, D, NH, HD = 2, 2048, 1024, 16, 64
ROPE_BASE = 10000.0
N_CORES = 8
HPC = NH // N_CORES          # heads per core = 2
M = B * L                    # 4096 tokens
MCH = 512                    # m-chunk (proj free dim)
NMC = M // MCH               # 8
KT = D // 128                # 8 contraction tiles
QB = 512                     # q chunk in attention
KB = 128                     # k block in attention
NKB = L // KB                # 16
NQC = L // QB                # 4

fp16 = mybir.dt.float16
fp32 = mybir.dt.float32

_NC = None


def _build_nc():
    nc = bacc.Bacc("TRN2", target_bir_lowering=False, debug=False,
                   num_devices=N_CORES)

    xT = nc.dram_tensor("xT", [D, M], fp16, kind="ExternalInput").ap()
    wqkvT = nc.dram_tensor("wqkvT", [D, 384], fp16, kind="ExternalInput").ap()
    woutT = nc.dram_tensor("woutT", [D, D], fp16, kind="ExternalInput").ap()
    cosT = nc.dram_tensor("cosT", [128, M], fp16, kind="ExternalInput").ap()
    sinT = nc.dram_tensor("sinT", [128, M], fp16, kind="ExternalInput").ap()
    out = nc.dram_tensor("out", [D, MCH], fp32, kind="ExternalOutput").ap()

    cc_in = nc.dram_tensor("cc_in", [D, MCH], fp16)
    cc_out = nc.dram_tensor("cc_out", [D, MCH], fp16)

    with tile.TileContext(nc) as tc:
        with tc.tile_pool(name="persist", bufs=1) as per, \
             tc.tile_pool(name="weights", bufs=1) as wp:
            # persistent SBUF: inputs, tables, Q/K units, V', masks
            xt = [per.tile([128, M], fp16, tag=f"xt{k}", name=f"xt{k}") for k in range(KT)]
            for k in range(KT):
                nc.sync.dma_start(xt[k][:], xT[k * 128:(k + 1) * 128, :])
            wq = [wp.tile([128, 384], fp16, tag=f"wq{k}", name=f"wq{k}") for k in range(KT)]
            for k in range(KT):
                nc.sync.dma_start(wq[k][:], wqkvT[k * 128:(k + 1) * 128, :])
            wo = [wp.tile([128, D], fp16, tag=f"wo{k}", name=f"wo{k}") for k in range(KT)]
            for k in range(KT):
                nc.sync.dma_start(wo[k][:], woutT[k * 128:(k + 1) * 128, :])
            cos_t = per.tile([128, M], fp16, tag="cos")
            sin_t = per.tile([128, M], fp16, tag="sin")
            nc.sync.dma_start(cos_t[:], cosT[:])
            nc.sync.dma_start(sin_t[:], sinT[:])

            # triangular 0/1 mask: keep where q-col >= k-row
            mask0 = per.tile([128, QB], fp16, tag="mask0")
            nc.gpsimd.memset(mask0[:], 1.0)
            nc.gpsimd.affine_select(
                out=mask0[:], in_=mask0[:], compare_op=mybir.AluOpType.is_ge,
                fill=0.0, base=0, channel_multiplier=-1, pattern=[[1, QB]],
            )

            # Q^T/K^T per batch: rows 0-63 head0, 64-127 head1;
            # cols 0:L = Q, L:2L = K
            qku = [per.tile([128, 2 * L], fp16, tag=f"qku{b}", name=f"qku{b}") for b in range(B)]
            # V' per batch: per k-tile 130 cols = [v_h0(64) | 1 | v_h1(64) | 1]
            vt = [per.tile([128, (L // 128) * 130], fp16, tag=f"vt{b}", name=f"vt{b}")
                  for b in range(B)]
            for b in range(B):
                nc.gpsimd.memset(vt[b][:], 1.0)

            # ---- Phase 1: QK projection + RoPE ----
            with tc.tile_pool(name="qkp_ps", bufs=2, space="PSUM") as qkps, \
                 tc.tile_pool(name="rope_sb", bufs=3) as rsb:
                for b_p in range(B):
                    for lh in range(HPC):
                        for mcb in range(NQC):
                            mc = b_p * NQC + mcb
                            qkp = qkps.tile([128, MCH], fp32, tag="qkp")
                            for k in range(KT):
                                nc.tensor.matmul(
                                    qkp[:], wq[k][:, lh * 128:(lh + 1) * 128],
                                    xt[k][:, mc * MCH:(mc + 1) * MCH],
                                    start=(k == 0), stop=(k == KT - 1))
                            qk16 = rsb.tile([128, MCH], fp16, tag="qk16")
                            nc.scalar.copy(qk16[:], qkp[:])
                            a_t = rsb.tile([128, MCH], fp16, tag="a")
                            c_t = rsb.tile([128, MCH], fp16, tag="c")
                            cs = slice(mc * MCH, (mc + 1) * MCH)
                            nc.vector.tensor_mul(a_t[:], qk16[:], cos_t[:, cs])
                            nc.vector.tensor_mul(c_t[:], qk16[:], sin_t[:, cs])
                            tmp = rsb.tile([128, MCH], fp16, tag="tmp")
                            for g in range(4):  # swap 32-row halves
                                src = (g ^ 1) * 32
                                nc.gpsimd.dma_start(
                                    tmp[g * 32:(g + 1) * 32, :],
                                    c_t[src:src + 32, :])
                            bcol = mcb * MCH
                            # head0: q rows 0-63, k rows 64-127
                            # head1: k rows 0-63, q rows 64-127
                            qrows = slice(0, 64) if lh == 0 else slice(64, 128)
                            krows = slice(64, 128) if lh == 0 else slice(0, 64)
                            drows = slice(lh * 64, (lh + 1) * 64)
                            nc.vector.tensor_add(
                                qku[b_p][drows, bcol:bcol + MCH],
                                a_t[qrows, :], tmp[qrows, :])
                            nc.vector.tensor_add(tmp[krows, :], a_t[krows, :],
                                                 tmp[krows, :])
                            nc.gpsimd.dma_start(
                                qku[b_p][drows, L + bcol:L + bcol + MCH],
                                tmp[krows, :])

            # ---- Phase 2: V projection (natural layout) ----
            with tc.tile_pool(name="v_ps", bufs=3, space="PSUM") as vps:
                for mt in range(M // 128):
                    vp = vps.tile([128, 128], fp32, tag="vp")
                    for k in range(KT):
                        nc.tensor.matmul(
                            vp[:], xt[k][:, mt * 128:(mt + 1) * 128],
                            wq[k][:, 256:384],
                            start=(k == 0), stop=(k == KT - 1))
                    b_, kt_ = mt // (L // 128), mt % (L // 128)
                    # strided dst: two 64-col blocks at +0 and +65
                    dst = vt[b_][:, kt_ * 130:kt_ * 130 + 130]
                    dst = dst.rearrange("p (g c) -> p g c", g=2)[:, :, 0:64]
                    nc.scalar.copy(dst, vp[:].rearrange("p (g c) -> p g c", g=2))

            # ---- Phase 3: attention per (batch, head) ----
            # Two heads of a batch interleaved; q-chunks in pairs so hacc
            # needs 2 banks/head and ST pairs share one [128,1024] tile
            # with a single exp per (head, ki).
            with tc.tile_pool(name="st_ps", bufs=3, space="PSUM") as stps, \
                 tc.tile_pool(name="h_ps", bufs=2, space="PSUM") as hps, \
                 tc.tile_pool(name="att_sb", bufs=4) as asb, \
                 tc.tile_pool(name="norm_sb", bufs=4) as nsb:
                for b_ in range(B):
                    for lh in range(HPC):
                      rows = slice(lh * 64, (lh + 1) * 64)
                      for pas in range(2):
                        qcs = (2 * pas, 2 * pas + 1)
                        kmax = (qcs[1] + 1) * (QB // KB)
                        hacc = {}
                        for qc in qcs:
                            hacc[qc] = hps.tile(
                                [65, QB], fp32, tag="hacc", name="hacc")
                        prev = None  # software pipeline: PV lags ST/exp by 1
                        for ki in range(kmax):
                            qlo = max(qcs[0], ki // (QB // KB))
                            pofs = []
                            for qc in range(qlo, qcs[1] + 1):
                                diag = (qc == ki // (QB // KB))
                                off = (ki % (QB // KB)) * KB if diag else 0
                                w = QB - off
                                # fixed 512-stride slots so each matmul stays
                                # inside one PSUM bank; diag slack unread
                                pofs.append((qc, (qc - qlo) * QB, w,
                                             qc * QB + off, off))
                            p0 = (qcs[1] + 1 - qlo) * QB
                            st = stps.tile([128, p0], fp32, tag="st")
                            pt = asb.tile([128, p0], fp16, tag="pt")
                            for qc, ps, w, qs, off in pofs:
                                nc.tensor.matmul(
                                    st[:, ps:ps + w],
                                    qku[b_][rows,
                                            L + ki * KB:L + (ki + 1) * KB],
                                    qku[b_][rows, qs:qs + w],
                                    start=True, stop=True)
                            nc.scalar.activation(
                                pt[:], st[:],
                                mybir.ActivationFunctionType.Exp)
                            if ki // (QB // KB) == qlo:
                                w0 = pofs[0][2]
                                nc.vector.tensor_mul(
                                    pt[:, 0:w0], pt[:, 0:w0], mask0[:, 0:w0])
                            if prev is not None:
                                kp, ppofs, ppt = prev
                                vsl = vt[b_][:, kp * 130 + lh * 65:
                                             kp * 130 + lh * 65 + 65]
                                for qc, ps, w, qs, off in ppofs:
                                    nc.tensor.matmul(
                                        hacc[qc][:, off:off + w], vsl,
                                        ppt[:, ps:ps + w],
                                        start=(kp == 0),
                                        stop=(kp == (qc + 1) * (QB // KB) - 1))
                            prev = (ki, pofs, pt)
                        kp, ppofs, ppt = prev
                        vsl = vt[b_][:, kp * 130 + lh * 65:
                                     kp * 130 + lh * 65 + 65]
                        for qc, ps, w, qs, off in ppofs:
                            nc.tensor.matmul(
                                hacc[qc][:, off:off + w], vsl,
                                ppt[:, ps:ps + w],
                                start=(kp == 0),
                                stop=(kp == (qc + 1) * (QB // KB) - 1))
                        # normalize + ship to cc_in
                        for qc in qcs:
                                ha = hacc[qc]
                                dsb = nsb.tile([1, QB], fp32, tag="dsb")
                                nc.scalar.copy(dsb[:], ha[64:65, :])
                                recip = nsb.tile([1, QB], fp32, tag="recip")
                                nc.vector.reciprocal_approx_fast(
                                    recip[:], dsb[:])
                                rb = nsb.tile([64, QB], fp32, tag="rb")
                                nc.gpsimd.partition_broadcast(rb[:], recip[:])
                                ht = nsb.tile([64, QB], fp16, tag="ht")
                                nc.vector.tensor_mul(
                                    ht[:], ha[0:64, :], rb[:])
                                j = b_ * NQC + qc
                                nc.sync.dma_start(
                                    cc_in.ap()[j * 128 + lh * 64:
                                               j * 128 + lh * 64 + 64, :],
                                    ht[:])

            # ---- Phase 4: AllToAll + output projection ----
            nc.gpsimd.collective_compute(
                "AllToAll", mybir.AluOpType.bypass,
                replica_groups=[list(range(N_CORES))],
                ins=[cc_in.ap().opt()], outs=[cc_out.ap().opt()],
            )
            with tc.tile_pool(name="op_ps", bufs=2, space="PSUM") as ops, \
                 tc.tile_pool(name="op_sb", bufs=3) as osb:
                htf = [osb.tile([128, MCH], fp16, tag=f"htf{k}", name=f"htf{k}")
                       for k in range(KT)]
                for k in range(KT):
                    nc.sync.dma_start(htf[k][:],
                                      cc_out.ap()[k * 128:(k + 1) * 128, :])
                for eb in range(KT):
                    op = ops.tile([128, MCH], fp32, tag="op")
                    for k in range(KT):
                        nc.tensor.matmul(
                            op[:], wo[k][:, eb * 128:(eb + 1) * 128], htf[k][:],
                            start=(k == 0), stop=(k == KT - 1))
                    ot = osb.tile([128, MCH], fp32, tag="ot")
                    nc.scalar.copy(ot[:], op[:])
                    nc.sync.dma_start(out[eb * 128:(eb + 1) * 128, :], ot[:])

    nc.compile()
    return nc


def _host_inputs(x, Wqkv, Wout):
    """Build the 8 per-core input maps (all fp16)."""
    x = np.asarray(x, dtype=np.float32)
    Wqkv = np.asarray(Wqkv, dtype=np.float32)
    Wout = np.asarray(Wout, dtype=np.float32)

    xT = np.ascontiguousarray(x.reshape(M, D).T).astype(np.float16)
    woutT = np.ascontiguousarray(Wout.T).astype(np.float16)

    scale = HD ** -0.5
    inv = ROPE_BASE ** (-np.arange(32, dtype=np.float64) / 32.0)
    l = np.arange(L, dtype=np.float64)
    ang = l[None, :] * inv[:, None]                      # [32, L]
    cos32 = np.cos(ang)
    sin32 = np.sin(ang)
    cosT = np.tile(cos32, (4, B)).astype(np.float16)     # [128, M]
    sgn = np.repeat([1.0, -1.0, 1.0, -1.0], 32)[:, None]
    sinT = (np.tile(sin32, (4, B)) * sgn).astype(np.float16)

    in_maps = []
    for c in range(N_CORES):
        a = HPC * c
        cols = []
        cols.append(Wqkv[HD * a:HD * (a + 1), :] * scale)          # q_a
        cols.append(Wqkv[D + HD * a:D + HD * (a + 1), :])          # k_a
        cols.append(Wqkv[D + HD * (a + 1):D + HD * (a + 2), :])    # k_{a+1}
        cols.append(Wqkv[HD * (a + 1):HD * (a + 2), :] * scale)    # q_{a+1}
        cols.append(Wqkv[2 * D + HD * a:2 * D + HD * (a + 1), :])  # v_a
        cols.append(Wqkv[2 * D + HD * (a + 1):2 * D + HD * (a + 2), :])
        wqkvT = np.ascontiguousarray(np.concatenate(cols, 0).T).astype(np.float16)
        in_maps.append({"xT": xT, "wqkvT": wqkvT, "woutT": woutT,
                        "cosT": cosT, "sinT": sinT})
    return in_maps


def kernel(x, Wqkv, Wout, _trace=False):
    global _NC
    if _NC is None:
        _NC = _build_nc()
    in_maps = _host_inputs(x, Wqkv, Wout)
    res = run_bass_kernel_spmd(_NC, in_maps, core_ids=list(range(N_CORES)),
                               trace=_trace)
    outT = np.concatenate([res.results[c]["out"] for c in range(N_CORES)],
                          axis=1)                        # [D, M]
    full = outT.T.reshape(B, L, D).astype(np.float32)
    if _trace:
        kernel.last_results = res
    return full
